# revision 31
# baseline (speedup 1.0000x reference)
"""Self-contained TRN2 Bass kernel for the 16-head MHA problem.

kernel(**inputs) takes FULL inputs (x [4,2048,1024], Wq/Wk/Wv [16,1024,64],
Wo [1024,1024], bo [1024]) and returns the FULL output [4,2048,1024] f32.

Sharding over 8 NeuronCores: core c handles batch b = c//2 and head group
g = c%2 (8 of 16 heads) — tensor parallel over heads with the output
projection's input dim sharded; the 2-way partial-sum reduce per batch and
the bias add happen host-side on the gathered results.
"""
import sys

for _p in ("/opt/trn_rl_repo",):
    if _p not in sys.path:
        sys.path.insert(0, _p)

import numpy as np
import concourse.bass as bass
import concourse.mybir as mybir
from concourse import bacc
from concourse import library_config
from concourse.bass import ts, ds
from concourse.tile import TileContext
from concourse.vector_clock import ScopedClock
from concourse import bass_utils

F32 = mybir.dt.float32
BF16 = mybir.dt.bfloat16
AF = mybir.ActivationFunctionType

NUM_HEADS = 16
EMB = 1024
HEAD = 64
SEQ = 2048
BATCH = 4
N_CORES = 8


class TC(TileContext):
    """TileContext whose final drain splits its sem waits across SP NOPs —
    the CTRL instruction encoding holds only one wait and this env's Tile
    puts the whole global clock on the tail drain."""

    def _drain_and_barrier(self, tick_clock, wait_clock):
        nc = self.nc
        dummy = nc.sync.nop(nofuse=True)
        wait_clock.add_sem_waits(dummy.ins, ScopedClock({None: tick_clock.global_clock}))
        si = dummy.ins.sync_info
        waits = list(si.on_wait) if si is not None else []
        if len(waits) > 1:
            si.on_wait = waits[:1]
            sem_by_name = {h.name: h for h in self.sems.allocated().values()}
            for w in waits[1:]:
                nop = nc.sync.nop(nofuse=True)
                nop._wait_ge(sem_by_name[w.ant_name], w.wait_value)
        nc.sync.drain()
        nc.all_engine_barrier()
        popped = nc._tile_sem_poison_stack.pop()
        assert popped is self._sem_poison
        nc.clear_and_free_semaphores(list(self.sems.allocated().values()))
        nc.all_engine_barrier()


def build_mha_nc(S=SEQ, E=EMB, D=HEAD, H=NUM_HEADS // 2):
    """Single-core SPMD program; H = heads per core (pair-packed).

    Fully transposed formulation:
      xT (host-transposed), qT/kT per pair (q pre-scaled by 1/sqrt(D) via
      host-scaled Wq), scoresT = kT-slice.T @ qT-slice. The two heads of a
      pair occupy PE row-tiles (0,0)/(64,0) (K=64, base partitions 0/64),
      and their score matmuls are emitted back-to-back so the PE runs them
      CONCURRENTLY (row tiling).
      attnT = exp(scoresT) (scores bounded, no max subtraction; one ACT op
      and one causal affine_select per head-tile),
      ctxT+denominator = [v|1].T @ attnT accumulated over k chunks,
      reciprocal of the denominator via DMA-reshape to [128, x] so all DVE
      lanes work, broadcast along d via gpsimd partition_broadcast (no PE),
      out = concatT-chunk.T @ WoT-chunk accumulated over head-dim chunks,
      written out as bf16 partials (host sums the pair + bias in f32).

    Scheduling structure: q/k projections for pair p+1 are interleaved
    into pair p's attention groups to fill PE gaps; ce accumulators are
    tagged by q-chunk parity so a group's normalization tail overlaps the
    next group's matmuls.
    """
    P = 128
    EC = E // P
    NQ = 512
    J = S // NQ
    KK = S // P
    NP = H // 2
    HD = H * D
    HC = HD // P
    NE = min(512, E)
    JE = E // NE
    RQ = NQ // 64          # reshape width for the two-row reciprocal trick

    nc = bacc.Bacc("TRN2", target_bir_lowering=False, debug=False)
    # host pre-casts x to bf16 and pre-arranges the (small) weights into
    # their on-chip layouts, halving the startup DMA bytes
    xT_in = nc.dram_tensor("xT_in", [P, EC, S], BF16, kind="ExternalInput")
    wq_in = nc.dram_tensor("wq_in", [P, NP, EC, 2 * D], BF16, kind="ExternalInput")
    wk_in = nc.dram_tensor("wk_in", [P, NP, EC, 2 * D], BF16, kind="ExternalInput")
    wv_in = nc.dram_tensor("wv_in", [P, EC, H, D], BF16, kind="ExternalInput")
    wo_in = nc.dram_tensor("wo_in", [P, HC, E], BF16, kind="ExternalInput")
    out_p = nc.dram_tensor("out_p", [S, E], BF16, kind="ExternalOutput")

    with TC(nc) as tc:
        with (
            tc.tile_pool(name="const", bufs=1) as cpool,
            tc.tile_pool(name="persist", bufs=1) as pers,
            tc.tile_pool(name="stage", bufs=5) as stg,
            tc.tile_pool(name="attn", bufs=6) as apool,
            tc.tile_pool(name="small", bufs=3) as spool,
            tc.tile_pool(name="psP", bufs=2, space="PSUM") as psP,
            tc.tile_pool(name="psC", bufs=1, space="PSUM") as psC,
        ):
            # memsets go on the Vector engine: the gpsimd LIBRARY_RELOAD for
            # partition_broadcast stalls the gpsimd queue ~10us, and the
            # warmup matmuls must not wait on it
            # full-K/M warm operands: K=1 warmups leave the HAM activity
            # monitor cold (phase A then runs at 1.2GHz); K=128/M=128 warms
            # reach full activity and unthrottle the PE clock
            warm_w = cpool.tile([P, P], mybir.dt.float16, tag="warmw")
            nc.vector.memset(warm_w[:], 0.0)
            warm_x = cpool.tile([P, NQ], mybir.dt.float16, tag="warmx")
            nc.vector.memset(warm_x[:], 0.0)
            # partition_broadcast lives in the loadable "attn" gpsimd library
            # (affine_select is a builtin and stays available)
            nc.gpsimd.load_library(library_config.attn)

            xT = pers.tile([P, EC, S], BF16, tag="xT")
            qT = pers.tile([P, NP, S], BF16, tag="qT")
            kT = pers.tile([P, NP, S], BF16, tag="kT")
            v_pad = pers.tile([P, KK, H, D + 1], BF16, tag="vp")
            woT = pers.tile([P, HC, E], BF16, tag="woT")
            concatT = pers.tile([P, NP, S], BF16, tag="concT")
            wq_bf = pers.tile([P, NP, EC, 2 * D], BF16, tag="wq")
            wk_bf = pers.tile([P, NP, EC, 2 * D], BF16, tag="wk")
            wv_bf = pers.tile([P, EC, H, D], BF16, tag="wv")

            nc.vector.memset(v_pad[:, :, :, D:D + 1], 1.0)

            # ---- Phase A: stream host-transposed bf16 xT in two waves of
            # per-ec slices (2KB contiguous DRAM runs per partition), with v
            # and pair-0 q/k projections fused in; weights land via single
            # pre-arranged DMAs queued between the waves ----
            # HAM warmup: keep the PE busy on throwaway matmuls while the
            # first DMAs stream, so real matmuls start at full clock
            warm_ps = psP.tile([P, 2, NQ], F32, tag="spair", name="warm_ps")
            for _w in range(14):
                nc.tensor.matmul(warm_ps[:, 0, :], warm_w[:], warm_x[:],
                                 start=True, stop=True)

            # DMA priority: v's inputs (wave0 + wv) first so emit_v can
            # start ~8us earlier; wq/wk are not needed until after v(0..3)
            HS = S // 2
            for ec in range(EC):
                nc.sync.dma_start(xT[:, ec, 0:HS], xT_in[:, ec, 0:HS])
            nc.sync.dma_start(wv_bf[:], wv_in[:])
            for ec in range(EC):
                nc.sync.dma_start(xT[:, ec, HS:S], xT_in[:, ec, HS:S])
            nc.sync.dma_start(wq_bf[:], wq_in[:])
            nc.sync.dma_start(wk_bf[:], wk_in[:])

            def emit_qk(p2, sc):
                # sc indexes NQ-wide chunks; scale is folded into Wq host-side
                for w_sb, dst in ((wq_bf, qT), (wk_bf, kT)):
                    acc = psP.tile(
                        [P, 2, NQ], F32, tag="spair",
                        name=f"qk_{p2}_{sc}_{0 if dst is qT else 1}",
                    )
                    for ec in range(EC):
                        nc.tensor.matmul(
                            acc[:, 0, :],
                            w_sb[:, p2, ec, :],
                            xT[:, ec, ts(sc, NQ)],
                            start=(ec == 0), stop=(ec == EC - 1),
                        )
                    nc.vector.tensor_copy(dst[:, p2, ts(sc, NQ)], acc[:, 0, :])

            def emit_v(sc):
                acc = psP.tile([P, 2, NQ], F32, tag="spair", name=f"vacc_{sc}")
                for ec in range(EC):
                    nc.tensor.matmul(
                        acc[:, 0, :],
                        xT[:, ec, ts(sc, P)],
                        wv_bf[:, ec, :, :].rearrange("p h d -> p (h d)"),
                        start=(ec == 0), stop=(ec == EC - 1),
                    )
                nc.vector.tensor_copy(
                    v_pad[:, sc, :, 0:D],
                    acc[:, 0, :].rearrange("p (h d) -> p h d", d=D),
                )

            def emit_out_chunks(j):
                for sc in range(4 * j, 4 * j + 4):
                    for n in range(JE):
                        acc = psP.tile([P, 2, NQ], F32, tag="spair",
                                       name=f"oacc_{sc}_{n}")
                        for hc in range(HC):
                            nc.tensor.matmul(
                                acc[:, 0, :],
                                concatT[:, hc, ts(sc, P)],
                                woT[:, hc, ts(n, NE)],
                                start=(hc == 0), stop=(hc == HC - 1),
                            )
                        ot = stg.tile([P, NE], BF16, tag="ostg")
                        nc.vector.tensor_copy(ot[:], acc[:, 0, :])
                        nc.sync.dma_start(out_p[ts(sc, P), ts(n, NE)], ot[:])

            def emit_normalize(p2, j, ce):
                # ~51-ULP approx reciprocal straight off the PSUM denominator
                # row (no DMA reshape round-trips: they cost ~2.3us each in
                # latency and stall the consumers)
                # NOTE: custom-DVE ops mis-read non-zero base partitions, so
                # the PSUM den row (partition 64) must be staged to partition
                # 0 by a plain copy before reciprocal_approx_fast
                dens = spool.tile([1, 2, NQ], F32, tag="dens",
                                  name=f"dens_{p2}_{j}")
                nc.vector.tensor_copy(dens[:], ce[ds(D, 1), :, :])
                recips = [
                    spool.tile([1, NQ], F32, tag=f"recip{hh}",
                               name=f"recip{hh}_{p2}_{j}")
                    for hh in range(2)
                ]
                for hh in range(2):
                    nc.vector.reciprocal_approx_fast(recips[hh][:],
                                                     dens[0:1, hh, :])
                for hh in range(2):
                    bc = spool.tile([D, NQ], F32, tag=f"bc{hh}",
                                    name=f"bc{hh}_{p2}_{j}")
                    nc.gpsimd.partition_broadcast(bc[:], recips[hh][:],
                                                  channels=D)
                    nc.vector.tensor_mul(
                        concatT[ds(hh * D, D), p2, ts(j, NQ)],
                        ce[0:D, hh, :], bc[:],
                    )

            pending = [None]

            def flush_pending():
                if pending[0] is not None:
                    p2x, jx, cex = pending[0]
                    emit_normalize(p2x, jx, cex)
                    pending[0] = None

            # crossed ctx order: pair A = (head0 k-lo on rows 0-63, head1
            # k-hi on rows 64-127), pair B = the opposite halves. Pair
            # members depend on the SAME at tile -> ready together -> the
            # scheduler keeps them adjacent and the PE overlaps them
            # (different row tiles, different PSUM banks). Same-bank halves
            # are 2 apart in pc order, so they never overlap (legal).
            CTX_SEQ = ((0, 0, True), (1, 1, True), (0, 1, False), (1, 0, False))

            def emit_group(p2, j):
                # previous group's normalize is emitted first; its parity ce
                # banks differ from this group's, so it overlaps freely
                flush_pending()
                heads = (2 * p2, 2 * p2 + 1)
                n_kk = min(KK, 4 * j + 4)
                ce = psC.tile([D + 1, 2, NQ], F32, tag=f"ce{j % 2}",
                              name=f"ce_{p2}_{j}")
                for i in range(n_kk):
                    t = i - 4 * j  # >= 0 -> diagonal (partial) tile
                    q0 = P * t if t > 0 else 0
                    nq = NQ - q0
                    # both heads' score matmuls: row tiles (0,0)/(64,0) into
                    # the two banks of one PSUM pair-tile, ready together ->
                    # the PE executes them concurrently
                    sps = psP.tile([P, 2, NQ], F32, tag="spair",
                                   name=f"s2_{p2}_{j}_{i}")
                    for hh in range(2):
                        nc.tensor.matmul(
                            sps[:, hh, 0:nq],
                            kT[ds(hh * D, D), p2, ts(i, P)],
                            qT[ds(hh * D, D), p2, ds(j * NQ + q0, nq)],
                            start=True, stop=True,
                        )
                    # one exp + one causal select covering both heads: all
                    # four ctx half-matmuls become ready at the same moment
                    at = apool.tile([P, 2, NQ], BF16, tag="at")
                    nc.scalar.activation(at[:, :, 0:nq], sps[:, :, 0:nq],
                                         AF.Exp)
                    if t >= 0:
                        nc.gpsimd.affine_select(
                            out=at[:, :, 0:nq], in_=at[:, :, 0:nq],
                            compare_op=mybir.AluOpType.is_ge,
                            fill=0.0, base=P * t - q0,
                            pattern=[[0, 2], [1, nq]], channel_multiplier=-1,
                        )
                    for hh in range(2):
                        nc.tensor.matmul(
                            ce[0:D + 1, hh, ds(q0, nq)],
                            v_pad[:, i, heads[hh], :],
                            at[:, hh, 0:nq],
                            start=(i == 0), stop=(i == n_kk - 1),
                        )
                pending[0] = (p2, j, ce)

            VDELAY = 2
            for sc in range(S // P):
                if sc == 2:
                    nc.sync.dma_start(woT[:], wo_in[:])
                if sc >= VDELAY:
                    emit_v(sc - VDELAY)
                if sc % 4 == 3:
                    emit_qk(0, sc // 4)
                if sc == 7:
                    emit_group(0, 0)
                if sc == 11:
                    emit_group(0, 1)
            for sc in range(S // P - VDELAY, S // P):
                emit_v(sc)

            # ---- remaining attention groups (pair 0 groups 0-1 were emitted
            # inside the x loop); next pair's q/k interleaved ----
            for p2 in range(NP):
                for j in range(J):
                    if p2 == 0 and j < 2:
                        emit_qk(1, j)
                        continue
                    emit_group(p2, j)
                    if p2 + 1 < NP:
                        emit_qk(p2 + 1, j)
                    elif j > 1:
                        # pair 3: emit out chunks AFTER the next group's
                        # score tiles so the shared psP ring never blocks
                        # scores behind normalize-gated out accumulators
                        emit_out_chunks(j - 1)
            # tail: normalize the final group FIRST (so its DVE/DMA chain
            # isn't queued behind the out-chunk copies), fill the PE with the
            # held-back j=0 out chunks, then drain the final j's out chunks
            p2x, jx, cex = pending[0]
            pending[0] = None
            emit_normalize(p2x, jx, cex)
            emit_out_chunks(0)
            emit_out_chunks(jx)

    nc.finalize()
    return nc


_NC_CACHE = {}


def _get_nc():
    key = "mha"
    if key not in _NC_CACHE:
        _NC_CACHE[key] = build_mha_nc()
    return _NC_CACHE[key]


def _arr_xT(xb, bf16):
    # [S, E] f32 -> [P, EC, S] bf16 with xT[p, ec, s] = x[s, ec*128+p]
    P, S, E = 128, xb.shape[0], xb.shape[1]
    xt = xb.astype(bf16).T.reshape(E // P, P, S)
    return np.ascontiguousarray(xt.transpose(1, 0, 2))


def _arr_wqk(w, bf16):
    # [H, E, D] -> [P, NP, EC, 2*D] pair-packed lhsT layout
    H, E, D = w.shape
    P = 128
    v = w.astype(bf16).reshape(H // 2, 2, E // P, P, D)
    v = v.transpose(3, 0, 2, 1, 4)  # [P, NP, EC, 2, D]
    return np.ascontiguousarray(v.reshape(P, H // 2, E // P, 2 * D))


def _arr_wv(w, bf16):
    # [H, E, D] -> [P, EC, H, D]
    H, E, D = w.shape
    P = 128
    v = w.astype(bf16).reshape(H, E // P, P, D)
    return np.ascontiguousarray(v.transpose(2, 1, 0, 3))


def _arr_wo(w, bf16):
    # [E, HD] -> [P, HC, E] with woT[p, hc, e] = Wo[e, hc*128+p]
    E, HD = w.shape
    P = 128
    v = w.astype(bf16).T.reshape(HD // P, P, E)
    return np.ascontiguousarray(v.transpose(1, 0, 2))


def kernel(x, Wq, Wk, Wv, Wo, bo, _runner_kwargs=None):
    import ml_dtypes
    bf16 = ml_dtypes.bfloat16
    x = np.asarray(x, dtype=np.float32)
    Wq = np.asarray(Wq, dtype=np.float32)
    Wk = np.asarray(Wk, dtype=np.float32)
    Wv = np.asarray(Wv, dtype=np.float32)
    Wo = np.asarray(Wo, dtype=np.float32)
    bo = np.asarray(bo, dtype=np.float32)

    HPC = NUM_HEADS // 2  # heads per core
    HDS = HPC * HEAD      # concat-dim slice per core
    scale = HEAD ** -0.5

    nc = _get_nc()
    xbs = [_arr_xT(x[b], bf16) for b in range(BATCH)]
    wq_scaled = Wq * scale  # fold softmax scale into the q projection
    in_maps = []
    for c in range(N_CORES):
        b, g = c // 2, c % 2
        hs = slice(g * HPC, (g + 1) * HPC)
        in_maps.append({
            "xT_in": xbs[b],
            "wq_in": _arr_wqk(wq_scaled[hs], bf16),
            "wk_in": _arr_wqk(Wk[hs], bf16),
            "wv_in": _arr_wv(Wv[hs], bf16),
            "wo_in": _arr_wo(Wo[:, g * HDS:(g + 1) * HDS], bf16),
        })

    kw = dict(_runner_kwargs or {})
    res = bass_utils.run_bass_kernel_spmd(
        nc, in_maps, core_ids=list(range(N_CORES)), **kw
    )

    out = np.empty((BATCH, SEQ, EMB), dtype=np.float32)
    for b in range(BATCH):
        p0 = np.asarray(res.results[2 * b]["out_p"]).astype(np.float32)
        p1 = np.asarray(res.results[2 * b + 1]["out_p"]).astype(np.float32)
        out[b] = p0 + p1 + bo
    if kw.get("trace"):
        kernel.last_results = res
    return out


# revision 33
# speedup vs baseline: 1.0459x; 1.0459x over previous
"""Self-contained TRN2 Bass kernel for the 16-head MHA problem.

kernel(**inputs) takes FULL inputs (x [4,2048,1024], Wq/Wk/Wv [16,1024,64],
Wo [1024,1024], bo [1024]) and returns the FULL output [4,2048,1024] f32.

Sharding over 8 NeuronCores: core c handles batch b = c//2 and head group
g = c%2 (8 of 16 heads) — tensor parallel over heads with the output
projection's input dim sharded; the 2-way partial-sum reduce per batch and
the bias add happen host-side on the gathered results.
"""
import sys

for _p in ("/opt/trn_rl_repo",):
    if _p not in sys.path:
        sys.path.insert(0, _p)

import numpy as np
import concourse.bass as bass
import concourse.mybir as mybir
from concourse import bacc
from concourse import library_config
from concourse.bass import ts, ds
from concourse.tile import TileContext
from concourse.vector_clock import ScopedClock
from concourse import bass_utils

F32 = mybir.dt.float32
BF16 = mybir.dt.bfloat16
AF = mybir.ActivationFunctionType

NUM_HEADS = 16
EMB = 1024
HEAD = 64
SEQ = 2048
BATCH = 4
N_CORES = 8


class TC(TileContext):
    """TileContext whose final drain splits its sem waits across SP NOPs —
    the CTRL instruction encoding holds only one wait and this env's Tile
    puts the whole global clock on the tail drain."""

    def _drain_and_barrier(self, tick_clock, wait_clock):
        nc = self.nc
        dummy = nc.sync.nop(nofuse=True)
        wait_clock.add_sem_waits(dummy.ins, ScopedClock({None: tick_clock.global_clock}))
        si = dummy.ins.sync_info
        waits = list(si.on_wait) if si is not None else []
        if len(waits) > 1:
            si.on_wait = waits[:1]
            sem_by_name = {h.name: h for h in self.sems.allocated().values()}
            for w in waits[1:]:
                nop = nc.sync.nop(nofuse=True)
                nop._wait_ge(sem_by_name[w.ant_name], w.wait_value)
        nc.sync.drain()
        nc.all_engine_barrier()
        popped = nc._tile_sem_poison_stack.pop()
        assert popped is self._sem_poison
        nc.clear_and_free_semaphores(list(self.sems.allocated().values()))
        nc.all_engine_barrier()


def build_mha_nc(S=SEQ, E=EMB, D=HEAD, H=NUM_HEADS // 2):
    """Single-core SPMD program; H = heads per core (pair-packed).

    Fully transposed formulation:
      xT (host-transposed), qT/kT per pair (q pre-scaled by 1/sqrt(D) via
      host-scaled Wq), scoresT = kT-slice.T @ qT-slice. The two heads of a
      pair occupy PE row-tiles (0,0)/(64,0) (K=64, base partitions 0/64),
      and their score matmuls are emitted back-to-back so the PE runs them
      CONCURRENTLY (row tiling).
      attnT = exp(scoresT) (scores bounded, no max subtraction; one ACT op
      and one causal affine_select per head-tile),
      ctxT+denominator = [v|1].T @ attnT accumulated over k chunks,
      reciprocal of the denominator via DMA-reshape to [128, x] so all DVE
      lanes work, broadcast along d via gpsimd partition_broadcast (no PE),
      out = concatT-chunk.T @ WoT-chunk accumulated over head-dim chunks,
      written out as bf16 partials (host sums the pair + bias in f32).

    Scheduling structure: q/k projections for pair p+1 are interleaved
    into pair p's attention groups to fill PE gaps; ce accumulators are
    tagged by q-chunk parity so a group's normalization tail overlaps the
    next group's matmuls.
    """
    P = 128
    EC = E // P
    NQ = 512
    J = S // NQ
    KK = S // P
    NP = H // 2
    HD = H * D
    HC = HD // P
    NE = min(512, E)
    JE = E // NE
    RQ = NQ // 64          # reshape width for the two-row reciprocal trick

    nc = bacc.Bacc("TRN2", target_bir_lowering=False, debug=False)
    # host pre-casts x to bf16 and pre-arranges the (small) weights into
    # their on-chip layouts, halving the startup DMA bytes
    xT_in = nc.dram_tensor("xT_in", [P, EC, S], BF16, kind="ExternalInput")
    wq_in = nc.dram_tensor("wq_in", [P, NP, EC, 2 * D], BF16, kind="ExternalInput")
    wk_in = nc.dram_tensor("wk_in", [P, NP, EC, 2 * D], BF16, kind="ExternalInput")
    wv_in = nc.dram_tensor("wv_in", [P, EC, H, D], BF16, kind="ExternalInput")
    wo_in = nc.dram_tensor("wo_in", [P, HC, E], BF16, kind="ExternalInput")
    out_p = nc.dram_tensor("out_p", [S, E], BF16, kind="ExternalOutput")

    with TC(nc) as tc:
        with (
            tc.tile_pool(name="const", bufs=1) as cpool,
            tc.tile_pool(name="persist", bufs=1) as pers,
            tc.tile_pool(name="stage", bufs=5) as stg,
            tc.tile_pool(name="attn", bufs=6) as apool,
            tc.tile_pool(name="small", bufs=3) as spool,
            tc.tile_pool(name="psP", bufs=2, space="PSUM") as psP,
            tc.tile_pool(name="psC", bufs=1, space="PSUM") as psC,
        ):
            # memsets go on the Vector engine: the gpsimd LIBRARY_RELOAD for
            # partition_broadcast stalls the gpsimd queue ~10us, and the
            # warmup matmuls must not wait on it
            # full-K/M warm operands: K=1 warmups leave the HAM activity
            # monitor cold (phase A then runs at 1.2GHz); K=128/M=128 warms
            # reach full activity and unthrottle the PE clock
            warm_w = cpool.tile([P, P], mybir.dt.float16, tag="warmw")
            nc.vector.memset(warm_w[:], 0.0)
            warm_x = cpool.tile([P, NQ], mybir.dt.float16, tag="warmx")
            nc.vector.memset(warm_x[:], 0.0)
            # partition_broadcast lives in the loadable "attn" gpsimd library
            # (affine_select is a builtin and stays available)
            nc.gpsimd.load_library(library_config.attn)

            xT = pers.tile([P, EC, S], BF16, tag="xT")
            qT = pers.tile([P, NP, S], BF16, tag="qT")
            kT = pers.tile([P, NP, S], BF16, tag="kT")
            v_pad = pers.tile([P, KK, H, D + 1], BF16, tag="vp")
            woT = pers.tile([P, HC, E], BF16, tag="woT")
            concatT = pers.tile([P, NP, S], BF16, tag="concT")
            wq_bf = pers.tile([P, NP, EC, 2 * D], BF16, tag="wq")
            wk_bf = pers.tile([P, NP, EC, 2 * D], BF16, tag="wk")
            wv_bf = pers.tile([P, EC, H, D], BF16, tag="wv")

            nc.vector.memset(v_pad[:, :, :, D:D + 1], 1.0)

            # ---- Phase A: stream host-transposed bf16 xT in two waves of
            # per-ec slices (2KB contiguous DRAM runs per partition), with v
            # and pair-0 q/k projections fused in; weights land via single
            # pre-arranged DMAs queued between the waves ----
            # HAM warmup: keep the PE busy on throwaway matmuls while the
            # first DMAs stream, so real matmuls start at full clock
            warm_ps = psP.tile([P, 2, NQ], F32, tag="spair", name="warm_ps")
            for _w in range(24):
                nc.tensor.matmul(warm_ps[:, 0, :], warm_w[:], warm_x[:],
                                 start=True, stop=True)

            HS = S // 2
            for ec in range(EC):
                nc.sync.dma_start(xT[:, ec, 0:HS], xT_in[:, ec, 0:HS])
            nc.sync.dma_start(wv_bf[:], wv_in[:])
            nc.sync.dma_start(wq_bf[:], wq_in[:])
            nc.sync.dma_start(wk_bf[:], wk_in[:])
            for ec in range(EC):
                nc.sync.dma_start(xT[:, ec, HS:S], xT_in[:, ec, HS:S])

            def emit_qk(p2, sc):
                # sc indexes NQ-wide chunks; scale is folded into Wq host-side
                for w_sb, dst in ((wq_bf, qT), (wk_bf, kT)):
                    acc = psP.tile(
                        [P, 2, NQ], F32, tag="spair",
                        name=f"qk_{p2}_{sc}_{0 if dst is qT else 1}",
                    )
                    for ec in range(EC):
                        nc.tensor.matmul(
                            acc[:, 0, :],
                            w_sb[:, p2, ec, :],
                            xT[:, ec, ts(sc, NQ)],
                            start=(ec == 0), stop=(ec == EC - 1),
                        )
                    nc.vector.tensor_copy(dst[:, p2, ts(sc, NQ)], acc[:, 0, :])

            def emit_v(sc):
                acc = psP.tile([P, 2, NQ], F32, tag="spair", name=f"vacc_{sc}")
                for ec in range(EC):
                    nc.tensor.matmul(
                        acc[:, 0, :],
                        xT[:, ec, ts(sc, P)],
                        wv_bf[:, ec, :, :].rearrange("p h d -> p (h d)"),
                        start=(ec == 0), stop=(ec == EC - 1),
                    )
                nc.vector.tensor_copy(
                    v_pad[:, sc, :, 0:D],
                    acc[:, 0, :].rearrange("p (h d) -> p h d", d=D),
                )

            def emit_out_chunks(j):
                for sc in range(4 * j, 4 * j + 4):
                    for n in range(JE):
                        acc = psP.tile([P, 2, NQ], F32, tag="spair",
                                       name=f"oacc_{sc}_{n}")
                        for hc in range(HC):
                            nc.tensor.matmul(
                                acc[:, 0, :],
                                concatT[:, hc, ts(sc, P)],
                                woT[:, hc, ts(n, NE)],
                                start=(hc == 0), stop=(hc == HC - 1),
                            )
                        ot = stg.tile([P, NE], BF16, tag="ostg")
                        nc.vector.tensor_copy(ot[:], acc[:, 0, :])
                        nc.sync.dma_start(out_p[ts(sc, P), ts(n, NE)], ot[:])

            def emit_normalize(p2, j, ce):
                # ~51-ULP approx reciprocal straight off the PSUM denominator
                # row (no DMA reshape round-trips: they cost ~2.3us each in
                # latency and stall the consumers)
                # NOTE: custom-DVE ops mis-read non-zero base partitions, so
                # the PSUM den row (partition 64) must be staged to partition
                # 0 by a plain copy before reciprocal_approx_fast
                dens = spool.tile([1, 2, NQ], F32, tag="dens",
                                  name=f"dens_{p2}_{j}")
                nc.vector.tensor_copy(dens[:], ce[ds(D, 1), :, :])
                recips = [
                    spool.tile([1, NQ], F32, tag=f"recip{hh}",
                               name=f"recip{hh}_{p2}_{j}")
                    for hh in range(2)
                ]
                for hh in range(2):
                    nc.vector.reciprocal_approx_fast(recips[hh][:],
                                                     dens[0:1, hh, :])
                for hh in range(2):
                    bc = spool.tile([D, NQ], F32, tag=f"bc{hh}",
                                    name=f"bc{hh}_{p2}_{j}")
                    nc.gpsimd.partition_broadcast(bc[:], recips[hh][:],
                                                  channels=D)
                    nc.vector.tensor_mul(
                        concatT[ds(hh * D, D), p2, ts(j, NQ)],
                        ce[0:D, hh, :], bc[:],
                    )

            pending = [None]

            def flush_pending():
                if pending[0] is not None:
                    p2x, jx, cex = pending[0]
                    emit_normalize(p2x, jx, cex)
                    pending[0] = None

            # crossed ctx order: pair A = (head0 k-lo on rows 0-63, head1
            # k-hi on rows 64-127), pair B = the opposite halves. Pair
            # members depend on the SAME at tile -> ready together -> the
            # scheduler keeps them adjacent and the PE overlaps them
            # (different row tiles, different PSUM banks). Same-bank halves
            # are 2 apart in pc order, so they never overlap (legal).
            CTX_SEQ = ((0, 0, True), (1, 1, True), (0, 1, False), (1, 0, False))

            def emit_group(p2, j):
                # previous group's normalize is emitted first; its parity ce
                # banks differ from this group's, so it overlaps freely
                flush_pending()
                heads = (2 * p2, 2 * p2 + 1)
                n_kk = min(KK, 4 * j + 4)
                ce = psC.tile([D + 1, 2, NQ], F32, tag=f"ce{j % 2}",
                              name=f"ce_{p2}_{j}")
                for i in range(n_kk):
                    t = i - 4 * j  # >= 0 -> diagonal (partial) tile
                    q0 = P * t if t > 0 else 0
                    nq = NQ - q0
                    # both heads' score matmuls: row tiles (0,0)/(64,0) into
                    # the two banks of one PSUM pair-tile, ready together ->
                    # the PE executes them concurrently
                    sps = psP.tile([P, 2, NQ], F32, tag="spair",
                                   name=f"s2_{p2}_{j}_{i}")
                    for hh in range(2):
                        nc.tensor.matmul(
                            sps[:, hh, 0:nq],
                            kT[ds(hh * D, D), p2, ts(i, P)],
                            qT[ds(hh * D, D), p2, ds(j * NQ + q0, nq)],
                            start=True, stop=True,
                        )
                    # one exp + one causal select covering both heads: all
                    # four ctx half-matmuls become ready at the same moment
                    at = apool.tile([P, 2, NQ], BF16, tag="at")
                    nc.scalar.activation(at[:, :, 0:nq], sps[:, :, 0:nq],
                                         AF.Exp)
                    if t >= 0:
                        # the invalid causal triangle (r > c) only touches
                        # the first 128 columns (r <= 127), so the select
                        # covers [*, *, 0:128] instead of the whole tile
                        nsel = min(P, nq)
                        nc.gpsimd.affine_select(
                            out=at[:, :, 0:nsel], in_=at[:, :, 0:nsel],
                            compare_op=mybir.AluOpType.is_ge,
                            fill=0.0, base=P * t - q0,
                            pattern=[[0, 2], [1, nsel]], channel_multiplier=-1,
                        )
                    for hh in range(2):
                        nc.tensor.matmul(
                            ce[0:D + 1, hh, ds(q0, nq)],
                            v_pad[:, i, heads[hh], :],
                            at[:, hh, 0:nq],
                            start=(i == 0), stop=(i == n_kk - 1),
                        )
                pending[0] = (p2, j, ce)

            VDELAY = 2
            for sc in range(S // P):
                if sc == 2:
                    nc.sync.dma_start(woT[:], wo_in[:])
                if sc >= VDELAY:
                    emit_v(sc - VDELAY)
                if sc % 4 == 3:
                    emit_qk(0, sc // 4)
                if sc == 7:
                    emit_group(0, 0)
                if sc == 11:
                    emit_group(0, 1)
            for sc in range(S // P - VDELAY, S // P):
                emit_v(sc)

            # ---- remaining attention groups (pair 0 groups 0-1 were emitted
            # inside the x loop); next pair's q/k interleaved ----
            for p2 in range(NP):
                for j in range(J):
                    if p2 == 0 and j < 2:
                        emit_qk(1, j)
                        continue
                    emit_group(p2, j)
                    if p2 + 1 < NP:
                        emit_qk(p2 + 1, j)
                    elif j > 1:
                        # pair 3: emit out chunks AFTER the next group's
                        # score tiles so the shared psP ring never blocks
                        # scores behind normalize-gated out accumulators
                        emit_out_chunks(j - 1)
            # tail: normalize the final group FIRST (so its DVE/DMA chain
            # isn't queued behind the out-chunk copies), fill the PE with the
            # held-back j=0 out chunks, then drain the final j's out chunks
            p2x, jx, cex = pending[0]
            pending[0] = None
            emit_normalize(p2x, jx, cex)
            emit_out_chunks(0)
            emit_out_chunks(jx)

    nc.finalize()
    return nc


_NC_CACHE = {}


def _get_nc():
    key = "mha"
    if key not in _NC_CACHE:
        _NC_CACHE[key] = build_mha_nc()
    return _NC_CACHE[key]


def _arr_xT(xb, bf16):
    # [S, E] f32 -> [P, EC, S] bf16 with xT[p, ec, s] = x[s, ec*128+p]
    P, S, E = 128, xb.shape[0], xb.shape[1]
    xt = xb.astype(bf16).T.reshape(E // P, P, S)
    return np.ascontiguousarray(xt.transpose(1, 0, 2))


def _arr_wqk(w, bf16):
    # [H, E, D] -> [P, NP, EC, 2*D] pair-packed lhsT layout
    H, E, D = w.shape
    P = 128
    v = w.astype(bf16).reshape(H // 2, 2, E // P, P, D)
    v = v.transpose(3, 0, 2, 1, 4)  # [P, NP, EC, 2, D]
    return np.ascontiguousarray(v.reshape(P, H // 2, E // P, 2 * D))


def _arr_wv(w, bf16):
    # [H, E, D] -> [P, EC, H, D]
    H, E, D = w.shape
    P = 128
    v = w.astype(bf16).reshape(H, E // P, P, D)
    return np.ascontiguousarray(v.transpose(2, 1, 0, 3))


def _arr_wo(w, bf16):
    # [E, HD] -> [P, HC, E] with woT[p, hc, e] = Wo[e, hc*128+p]
    E, HD = w.shape
    P = 128
    v = w.astype(bf16).T.reshape(HD // P, P, E)
    return np.ascontiguousarray(v.transpose(1, 0, 2))


def kernel(x, Wq, Wk, Wv, Wo, bo, _runner_kwargs=None):
    import ml_dtypes
    bf16 = ml_dtypes.bfloat16
    x = np.asarray(x, dtype=np.float32)
    Wq = np.asarray(Wq, dtype=np.float32)
    Wk = np.asarray(Wk, dtype=np.float32)
    Wv = np.asarray(Wv, dtype=np.float32)
    Wo = np.asarray(Wo, dtype=np.float32)
    bo = np.asarray(bo, dtype=np.float32)

    HPC = NUM_HEADS // 2  # heads per core
    HDS = HPC * HEAD      # concat-dim slice per core
    scale = HEAD ** -0.5

    nc = _get_nc()
    xbs = [_arr_xT(x[b], bf16) for b in range(BATCH)]
    wq_scaled = Wq * scale  # fold softmax scale into the q projection
    in_maps = []
    for c in range(N_CORES):
        b, g = c // 2, c % 2
        hs = slice(g * HPC, (g + 1) * HPC)
        in_maps.append({
            "xT_in": xbs[b],
            "wq_in": _arr_wqk(wq_scaled[hs], bf16),
            "wk_in": _arr_wqk(Wk[hs], bf16),
            "wv_in": _arr_wv(Wv[hs], bf16),
            "wo_in": _arr_wo(Wo[:, g * HDS:(g + 1) * HDS], bf16),
        })

    kw = dict(_runner_kwargs or {})
    res = bass_utils.run_bass_kernel_spmd(
        nc, in_maps, core_ids=list(range(N_CORES)), **kw
    )

    out = np.empty((BATCH, SEQ, EMB), dtype=np.float32)
    for b in range(BATCH):
        p0 = np.asarray(res.results[2 * b]["out_p"]).astype(np.float32)
        p1 = np.asarray(res.results[2 * b + 1]["out_p"]).astype(np.float32)
        out[b] = p0 + p1 + bo
    if kw.get("trace"):
        kernel.last_results = res
    return out
